# revision 2
# baseline (speedup 1.0000x reference)
"""Trainium2 Bass kernel for nn_Conv3_5738076307876.

Math: the reference's diagonal-embed + Conv3d collapses to a 2D conv:
  out[b, o, d, h, w] = sum_{i,kh,kw} x[b,i,h+kh-2,w+kw-2] * W[o,i,i-d+2,kh,kw]
                       + bias[o]           (terms with |i-d|>2 vanish)
i.e. a 5x5 conv2d with 10 input channels and 100 output channels (o,d).

Device scheme (per core, data-parallel over batch: 4 images/core):
  - host pre-pads each image to a flat 132x132 bf16 buffer (zeros baked in,
    16-element zero tail to SLAB_F=17440) and uploads it; one flat DMA per
    image fills slab group 0, then 9 flat shifted SBUF->SBUF copies build
    groups 1..9 (group g = (kwg*5+kh) holds the image shifted kh*132+kwg).
  - 3 accumulating matmuls per 512-pixel PSUM tile: contraction packed as
    (kwg, kh, i) -> K=100/100/50, kw remainder (0/2/4) via free-dim offset.
    (10 groups x 3 offsets is provably the minimum matmul count: covering
    the 5x5 tap grid by G (+) F with |F|=3 needs |G|>=10.)
  - bias is applied during PSUM->SBUF staging (ACT: activation-Identity
    with per-partition bias; DVE: tensor_scalar_add), which also casts the
    result to bf16. One engine per half-image so each out-DMA waits on a
    single semaphore. Output leaves the device as bf16 (rel-err budget 2e-2
    dwarfs bf16 rounding); host casts back to fp32.
  - ~48 dependency-free warmup matmuls on a scratch tile keep the PE busy
    while slab 0 builds, so the Tensor engine is at its full p-state clock
    when real work starts (cold PE runs 0.65/1.2 GHz for the first ~3us).
  - slab builds + output DMAs all ride the SP HWDGE ring in program order:
    build(0), build(1), then build(bb+2) interleaved between image bb's two
    half-image output DMAs, which keeps the DMA engines fed without letting
    outputs starve slab builds (or vice versa).
"""

import os
import sys

for _p in ("/root/.axon_site/_ro/trn_rl_repo", "/opt/trn_rl_repo"):
    if os.path.isdir(_p) and _p not in sys.path:
        sys.path.append(_p)

import numpy as np
import ml_dtypes

import concourse.bass as bass
import concourse.bacc as bacc
import concourse.mybir as mybir
import concourse.tile as tile
from concourse.bass_utils import run_bass_kernel_spmd

N_CORES = 8
B, C, H, W_DIM, KS = 32, 10, 128, 128, 5
BC = B // N_CORES          # images per core
OD = 100                   # (o, d) output pairs
PADW = 132                 # padded row length
FLAT = PADW * PADW         # 17424
SLAB_F = 17440             # slab free size (32B-aligned)
ROWS_PER_TILE = 4          # 4*128 = 512 = one PSUM bank of fp32
TILES = H // ROWS_PER_TILE  # 32 tiles per image
HTILES = 16                # tiles per staged half-image
WARMUP_MMS = 48            # PE p-state warmup matmuls

_F32 = mybir.dt.float32
_BF16 = mybir.dt.bfloat16

# shift of group g (g = kwg*5 + kh)
_SHIFTS = [kh * PADW + kwg for kwg in (0, 1) for kh in range(KS)]


def _pack_weights(W: np.ndarray) -> np.ndarray:
    """wt [100, 300] bf16; wt[p, j*100+od] = lhsT_j[p, od].

    p = (kwg*5+kh)*10 + i, kw = kwg + 2*j (j in 0..2), od = o*10+d.
    """
    Weff = np.zeros((10, 10, 10, KS, KS), np.float32)  # [o, d, i, kh, kw]
    for d in range(10):
        for i in range(max(0, d - 2), min(10, d + 3)):
            Weff[:, d, i] = W[:, i, i - d + 2]
    Weff = Weff.reshape(OD, 10, KS, KS)
    wt = np.zeros((100, 300), np.float32)
    for j in range(3):
        for kwg in range(2):
            kw = kwg + 2 * j
            if kw > 4:
                continue
            for kh in range(KS):
                for i in range(10):
                    p = (kwg * 5 + kh) * 10 + i
                    wt[p, j * 100:(j + 1) * 100] = Weff[:, i, kh, kw]
    return wt.astype(ml_dtypes.bfloat16)


def _pack_inputs(x: np.ndarray) -> np.ndarray:
    """[B, C, SLAB_F] bf16: zero-padded 132x132 image, flattened, zero tail."""
    xp = np.zeros((B, C, SLAB_F), ml_dtypes.bfloat16)
    v = xp[:, :, :FLAT].reshape(B, C, PADW, PADW)
    v[:, :, 2:130, 2:130] = x.astype(ml_dtypes.bfloat16)
    return xp


def _build_nc() -> bass.Bass:
    nc = bacc.Bacc()
    x_d = nc.dram_tensor("xp", [BC, C, SLAB_F], _BF16, kind="ExternalInput")
    wt_d = nc.dram_tensor("wt", [100, 300], _BF16, kind="ExternalInput")
    bias_d = nc.dram_tensor("bias", [OD, 1], _F32, kind="ExternalInput")
    out_d = nc.dram_tensor("out", [BC, OD, H, W_DIM], _BF16,
                           kind="ExternalOutput")

    with tile.TileContext(nc) as tc:
        with (
            tc.tile_pool(name="const", bufs=1) as const_pool,
            tc.tile_pool(name="slab", bufs=1) as slab_pool,
            tc.tile_pool(name="stage", bufs=3) as stage_pool,
            tc.tile_pool(name="psum", bufs=8, space="PSUM") as psum_pool,
        ):
            wt = const_pool.tile([100, 300], _BF16)
            nc.sync.dma_start(wt[:, :], wt_d[:, :])
            bias = const_pool.tile([OD, 1], _F32)
            nc.sync.dma_start(bias[:, :], bias_d[:, :])

            # PE warmup: scratch matmuls with no data deps ramp the clock
            # while slab 0 builds. Zero-init so CoreSim never sees
            # uninitialized reads.
            scratch = const_pool.tile([128, 512], _BF16)
            nc.vector.memset(scratch[:, :], 0.0)

            slabs = [
                slab_pool.tile([OD, SLAB_F], _BF16,
                               name=f"slab{i}", tag=f"slab{i}")
                for i in range(BC)
            ]

            def build(bb):
                slab = slabs[bb]
                nc.sync.dma_start(slab[0:10, 0:SLAB_F], x_d[bb])
                # flat dup: group g <- group 0 shifted by s_g (all read only
                # group 0, so each dup waits on a single DMA semaphore)
                for g in range(1, 10):
                    sg = _SHIFTS[g]
                    nc.sync.dma_start(
                        slab[10 * g:10 * g + 10, 0:SLAB_F - sg],
                        slab[0:10, sg:SLAB_F])

            build(0)
            build(1)

            for k in range(WARMUP_MMS):
                ps = psum_pool.tile([OD, 512], _F32)
                nc.tensor.matmul(ps[:, :], scratch[0:100, 0:100],
                                 scratch[0:100, 0:512],
                                 start=True, stop=True)

            for bb in range(BC):
                slab = slabs[bb]
                view = slab[0:OD, 0:FLAT].rearrange("p (h w) -> p h w", w=PADW)

                for half in range(2):
                    stage = stage_pool.tile([OD, HTILES * 512], _BF16)
                    for tt in range(HTILES):
                        t = half * HTILES + tt
                        r0 = ROWS_PER_TILE * t
                        ps = psum_pool.tile([OD, 512], _F32)
                        nc.tensor.matmul(
                            ps[:, :], wt[0:100, 0:100],
                            view[0:100, r0:r0 + 4, 0:128],
                            start=True, stop=False)
                        nc.tensor.matmul(
                            ps[:, :], wt[0:100, 100:200],
                            view[0:100, r0:r0 + 4, 2:130],
                            start=False, stop=False)
                        nc.tensor.matmul(
                            ps[:, :], wt[0:50, 200:300],
                            view[0:50, r0:r0 + 4, 4:132],
                            start=False, stop=True)
                        dst = stage[:, tt * 512:(tt + 1) * 512]
                        # one engine per half-image stage so the out-DMA
                        # waits on exactly one semaphore; bias + bf16 cast
                        # fused into the PSUM->SBUF copy
                        if half == 0:
                            nc.scalar.activation(
                                dst, ps[:, :],
                                mybir.ActivationFunctionType.Identity,
                                bias=bias[:, 0:1])
                        else:
                            nc.vector.tensor_scalar_add(
                                dst, ps[:, :], bias[:, 0:1])
                    h0 = half * HTILES * ROWS_PER_TILE
                    nc.sync.dma_start(
                        out_d[bb, :, h0:h0 + HTILES * ROWS_PER_TILE, :],
                        stage[:, :].rearrange("p (h w) -> p h w", w=W_DIM),
                    )
                    if half == 0 and bb + 2 < BC:
                        build(bb + 2)
    nc.finalize()
    return nc


_NC_CACHE = None


def kernel(x: np.ndarray, W: np.ndarray, b: np.ndarray) -> np.ndarray:
    global _NC_CACHE
    x = np.ascontiguousarray(np.asarray(x, dtype=np.float32))
    W = np.asarray(W, dtype=np.float32)
    b = np.asarray(b, dtype=np.float32)
    wt = _pack_weights(W)
    xp = _pack_inputs(x)
    bias = np.repeat(b, 10).reshape(OD, 1).astype(np.float32)

    if _NC_CACHE is None:
        _NC_CACHE = _build_nc()
    nc = _NC_CACHE

    core_ids = list(range(N_CORES))
    in_maps = [
        {"xp": xp[k * BC:(k + 1) * BC], "wt": wt, "bias": bias}
        for k in core_ids
    ]
    res = run_bass_kernel_spmd(nc, in_maps, core_ids)
    outs = [np.asarray(res.results[k]["out"]) for k in core_ids]
    full = np.concatenate(outs, axis=0)  # [32, 100, 128, 128] bf16
    return full.reshape(B, 10, 10, H, W_DIM).astype(np.float32)
